# revision 4
# baseline (speedup 1.0000x reference)
"""Bahdanau-attention Trainium2 kernel (8 NeuronCores, data-parallel over batch).

Problem shapes (full): decoder_hidden [64,1024], encoder_outputs [64,2048,1024],
W1_w/W2_w [1024,1024], W1_b/W2_b/v_w [1024].
Returns (context [64,1024], attn_weights [64,2048]) as float32, matching:
    enc_proj = enc @ W1 + b1
    dec_proj = dec @ W2 + b2
    e = tanh(enc_proj + dec_proj[:, None, :])
    scores = e @ v ; attn = softmax(scores, axis=S) ; context = attn @ enc

Sharding: batch 64 -> 8 per core, weights replicated, no collectives.
"""

import sys

for _p in ("/opt/trn_rl_repo", "/opt/pypackages"):
    if _p not in sys.path:
        sys.path.insert(0, _p)

import numpy as np

import concourse.bass as bass  # noqa: E402
import concourse.mybir as mybir  # noqa: E402
import concourse.tile as tile  # noqa: E402
from concourse import bacc  # noqa: E402
from concourse.bass_utils import run_bass_kernel_spmd  # noqa: E402

F32 = mybir.dt.float32
BF16 = mybir.dt.bfloat16
AF = mybir.ActivationFunctionType
ALU = mybir.AluOpType

NCORES = 8
B = 8        # batches per core
S = 2048
H = 1024
ST = 512     # tokens per s-tile
NST = S // ST          # 4 s-tiles per batch
HCH = H // 128         # 8 h-chunks
TPB = S // 128         # 16 token-chunks per batch


def build_kernel():
    nc = bacc.Bacc("TRN2", target_bir_lowering=False, debug=False)

    enc = nc.dram_tensor("encoder_outputs", [B, S, H], F32, kind="ExternalInput").ap()
    dec = nc.dram_tensor("decoder_hidden", [B, H], F32, kind="ExternalInput").ap()
    w1 = nc.dram_tensor("W1_w", [H, H], F32, kind="ExternalInput").ap()
    b1 = nc.dram_tensor("W1_b", [H], F32, kind="ExternalInput").ap()
    w2 = nc.dram_tensor("W2_w", [H, H], F32, kind="ExternalInput").ap()
    b2 = nc.dram_tensor("W2_b", [H], F32, kind="ExternalInput").ap()
    vw = nc.dram_tensor("v_w", [H], F32, kind="ExternalInput").ap()
    ctx_out = nc.dram_tensor("context", [B, H], F32, kind="ExternalOutput").ap()
    attn_out = nc.dram_tensor("attn", [B, S], F32, kind="ExternalOutput").ap()

    with tile.TileContext(nc) as tc:
        with (
            tc.tile_pool(name="singles", bufs=1) as singles,
            tc.tile_pool(name="nat", bufs=2) as nat_pool,
            tc.tile_pool(name="enct", bufs=3) as enct_pool,
            tc.tile_pool(name="esb", bufs=2) as esb_pool,
            tc.tile_pool(name="rows", bufs=2) as row_pool,
            tc.tile_pool(name="smalls", bufs=4) as small_pool,
            tc.tile_pool(name="stage", bufs=3, space="DRAM") as stage_pool,
            tc.tile_pool(name="ps_e", bufs=2, space="PSUM") as ps_e,
            tc.tile_pool(name="ps_s", bufs=2, space="PSUM") as ps_s,
            tc.tile_pool(name="ps_c", bufs=2, space="PSUM") as ps_c,
        ):
            # ---------------- weights / constants (one-time) ----------------
            # W1 as lhsT chunks: w1_sb[p, kc, m] = W1[kc*128+p, m]  (bf16)
            w1_sb = singles.tile([128, HCH, H], BF16)
            nc.gpsimd.dma_start(
                out=w1_sb, in_=w1.rearrange("(kc p) m -> p kc m", p=128)
            )
            w2_sb = singles.tile([128, HCH, H], BF16)
            nc.gpsimd.dma_start(
                out=w2_sb, in_=w2.rearrange("(kc p) m -> p kc m", p=128)
            )
            # v as per-chunk stationary columns: v_sb[p, c] = v[c*128+p]
            v_sb = singles.tile([128, HCH], BF16)
            nc.gpsimd.dma_start(out=v_sb, in_=vw.rearrange("(c p) -> p c", p=128))
            # bias = b1 + b2, chunk-major [128, HCH]
            b1_sb = singles.tile([128, HCH], F32)
            nc.gpsimd.dma_start(out=b1_sb, in_=b1.rearrange("(c p) -> p c", p=128))
            b2_sb = singles.tile([128, HCH], F32)
            nc.gpsimd.dma_start(out=b2_sb, in_=b2.rearrange("(c p) -> p c", p=128))
            bias_sb = singles.tile([128, HCH], F32)
            nc.vector.tensor_add(bias_sb, b1_sb, b2_sb)
            # dec^T chunks: dect[p, c, b] = dec[b, c*128+p]  (bf16 for matmul)
            dect_f = singles.tile([128, HCH, B], F32)
            dec_r = dec.rearrange("b (c p) -> p c b", p=128)
            for c in range(HCH):
                nc.gpsimd.dma_start(out=dect_f[:, c, :], in_=dec_r[:, c, :])
            dect = singles.tile([128, HCH, B], BF16)
            nc.vector.tensor_copy(dect, dect_f)

            # ---------------- D^T = (dec @ W2 + b1 + b2)^T  [128, HCH, B] ----
            d_sb = singles.tile([128, HCH, B], F32)
            for mc in range(HCH):
                ps_d = ps_e.tile([128, ST], F32, tag="epsum")
                for kc in range(HCH):
                    nc.tensor.matmul(
                        ps_d[:, :B],
                        lhsT=w2_sb[:, kc, mc * 128:(mc + 1) * 128],
                        rhs=dect[:, kc, :],
                        start=(kc == 0),
                        stop=(kc == HCH - 1),
                    )
                nc.vector.tensor_scalar(
                    d_sb[:, mc, :], ps_d[:, :B], bias_sb[:, mc:mc + 1], None, ALU.add
                )

            # ---------------- main loop over batches ----------------
            for b in range(B):
                # all 16 natural token-chunks of this batch stay resident
                bnat = nat_pool.tile([128, TPB, H], BF16)
                scores_row = row_pool.tile([1, S], F32, tag="scores")

                for st in range(NST):
                    s0 = st * ST
                    # cast-load natural tiles (f32 DRAM -> bf16 SBUF)
                    for t in range(ST // 128):
                        i16 = st * (ST // 128) + t
                        nc.gpsimd.dma_start(
                            out=bnat[:, i16, :],
                            in_=enc[b, s0 + t * 128: s0 + (t + 1) * 128, :],
                        )
                    # stage natural bf16 to DRAM for the transposed reload
                    stg = stage_pool.tile([ST, H], BF16)
                    nc.sync.dma_start(
                        out=stg.rearrange("(t p) h -> p t h", p=128),
                        in_=bnat[:, st * (ST // 128):(st + 1) * (ST // 128), :],
                    )
                    # transposed load: encT[p, hc, tok] = enc[b, s0+tok, hc*128+p]
                    enct = enct_pool.tile([128, HCH, ST], BF16)
                    for hc in range(HCH):
                        nc.sync.dma_start_transpose(
                            out=enct[:, hc, :], in_=stg[:, hc * 128:(hc + 1) * 128]
                        )

                    # W1 matmuls + fused bias/tanh
                    e_sb = esb_pool.tile([128, HCH, ST], BF16)
                    for mc in range(HCH):
                        ps = ps_e.tile([128, ST], F32, tag="epsum")
                        for kc in range(HCH):
                            nc.tensor.matmul(
                                ps,
                                lhsT=w1_sb[:, kc, mc * 128:(mc + 1) * 128],
                                rhs=enct[:, kc, :],
                                start=(kc == 0),
                                stop=(kc == HCH - 1),
                            )
                        nc.scalar.activation(
                            out=e_sb[:, mc, :], in_=ps, func=AF.Tanh,
                            bias=d_sb[:, mc, b:b + 1],
                        )

                    # scores for this s-tile: [1, ST]
                    ps_sc = ps_s.tile([1, ST], F32, tag="spsum")
                    for hc in range(HCH):
                        nc.tensor.matmul(
                            ps_sc,
                            lhsT=v_sb[:, hc:hc + 1],
                            rhs=e_sb[:, hc, :],
                            start=(hc == 0),
                            stop=(hc == HCH - 1),
                        )
                    nc.vector.tensor_copy(scores_row[:, s0:s0 + ST], ps_sc)

                # softmax pieces: exact, max-subtracted
                neg_max = small_pool.tile([1, 1], F32, tag="negmax")
                nc.vector.tensor_reduce(
                    neg_max, scores_row, mybir.AxisListType.X, ALU.max, negate=True
                )
                exp_row = row_pool.tile([1, S], F32, tag="exps")
                den = small_pool.tile([1, 1], F32, tag="den")
                nc.scalar.activation(
                    out=exp_row, in_=scores_row, func=AF.Exp, bias=neg_max,
                    accum_out=den,
                )
                rden = small_pool.tile([1, 1], F32, tag="rden")
                nc.vector.reciprocal(rden, den)

                # spray exp weights across partitions: wT[p, c] = exp_row[c*128+p]
                # (via DRAM bounce — SBUF partition dim can't absorb free bytes)
                wrow_d = stage_pool.tile([S], F32, tag="wrow")
                nc.sync.dma_start(out=wrow_d, in_=exp_row)
                wt_f = small_pool.tile([128, TPB], F32, tag="wtf")
                nc.sync.dma_start(
                    out=wt_f, in_=wrow_d.rearrange("(c p) -> p c", p=128)
                )
                wt = small_pool.tile([128, TPB], BF16, tag="wtb")
                nc.vector.tensor_copy(wt, wt_f)

                # context: ctx[h] = sum_tok w[tok] * enc[tok, h], per 512-col half
                ps_cx0 = ps_c.tile([1, ST], F32, tag="cpsum0")
                ps_cx1 = ps_c.tile([1, ST], F32, tag="cpsum1")
                ps_cx = [ps_cx0, ps_cx1]
                for c in range(TPB):
                    for nh in range(2):
                        nc.tensor.matmul(
                            ps_cx[nh],
                            lhsT=wt[:, c:c + 1],
                            rhs=bnat[:, c, nh * ST:(nh + 1) * ST],
                            start=(c == 0),
                            stop=(c == TPB - 1),
                        )
                ctx_sb = small_pool.tile([1, H], F32, tag="ctx")
                for nh in range(2):
                    nc.vector.tensor_scalar(
                        ctx_sb[:, nh * ST:(nh + 1) * ST], ps_cx[nh], rden, None,
                        ALU.mult,
                    )
                attn_sb = row_pool.tile([1, S], F32, tag="attn")
                nc.vector.tensor_scalar(attn_sb, exp_row, rden, None, ALU.mult)

                nc.sync.dma_start(out=ctx_out[b:b + 1, :], in_=ctx_sb)
                nc.sync.dma_start(out=attn_out[b:b + 1, :], in_=attn_sb)

    nc.compile()
    return nc


_NC = None


def _get_nc():
    global _NC
    if _NC is None:
        _NC = build_kernel()
    return _NC


def kernel(decoder_hidden, encoder_outputs, W1_w, W1_b, W2_w, W2_b, v_w):
    nc = _get_nc()
    fb = np.ascontiguousarray
    in_maps = []
    for i in range(NCORES):
        lo, hi = i * B, (i + 1) * B
        in_maps.append({
            "encoder_outputs": fb(encoder_outputs[lo:hi], dtype=np.float32),
            "decoder_hidden": fb(decoder_hidden[lo:hi], dtype=np.float32),
            "W1_w": fb(W1_w, dtype=np.float32),
            "W1_b": fb(W1_b, dtype=np.float32),
            "W2_w": fb(W2_w, dtype=np.float32),
            "W2_b": fb(W2_b, dtype=np.float32),
            "v_w": fb(v_w, dtype=np.float32),
        })
    res = run_bass_kernel_spmd(nc, in_maps, core_ids=list(range(NCORES)))
    results = res.results
    context = np.concatenate([r["context"] for r in results], axis=0)
    attn = np.concatenate([r["attn"] for r in results], axis=0)
    return context, attn
